# revision 16
# baseline (speedup 1.0000x reference)
"""Trainium2 Bass kernel: batched nearest-center (VQ codebook) one-hot assignment.

Computes, for each element x of the kept timesteps of y_true:
    idx = argmin_k |x - centers_k| ;  out = one_hot(idx, K)

Method (device side):
  The nearest center among K scalar centers is determined by which interval
  between sorted-center midpoints x falls into.  rank(x) = #{midpoints < x}
  is computed with 63 fused compare-accumulate passes (scalar_tensor_tensor:
  acc = (x > m_k) + acc).  The one-hot expansion in *original* center order
  is then a single is_equal pass per output chunk against a permuted iota
  (iota[j] = sorted-rank of original center j), using stride-0 broadcast APs.

Sharding: pure data parallel, batch B=8 across 8 NeuronCores.
Output is memory-bound: each core writes t_keep*C*F*K*4 = 67 MB.
"""

import functools
import sys
from contextlib import ExitStack

import numpy as np

for _p in ("/opt/trn_rl_repo",):
    if _p not in sys.path:
        sys.path.append(_p)

import concourse.bass as bass
import concourse.tile as tile
from concourse import bacc, mybir
from concourse.bass_utils import run_bass_kernel_spmd

P = 128          # SBUF partitions
K = 64           # number of centers
NCORES = 8

# trace flag poked by test harness; not used in grading path
TRACE = False
LAST_RESULTS = None


def _chunk_plan(E):
    """Split the per-partition free dim E into stt groups and is_equal chunks.

    Returns (groups, chunks): groups = [(off, len)] for the rank accumulation,
    chunks = [(off, len, engine)] for one-hot emission, engine in {"v", "g"}.
    """
    CE = 128
    while E % CE != 0:
        CE //= 2
    n_chunks = E // CE
    # first group small so the DMA pipeline starts early
    if E % 16 == 0 and E >= 1024:
        g0 = max(CE, E // 8)
        rest = E - g0
        g12 = rest // 2
        groups = [(0, g0), (g0, g12), (g0 + g12, rest - g12)]
    else:
        groups = [(0, E)]
    # all chunks on the vector engine: walrus allows a single sync-wait per
    # TT instruction, and gpsimd (Pool) is modeled out-of-order so its chunks
    # would need two waits (slot recycle + cross-engine data dep)
    chunks = [(i * CE, CE, "v") for i in range(n_chunks)]
    return groups, chunks


@functools.lru_cache(maxsize=4)
def _build(E):
    """Build the Bass program for per-core input [P, E+127] f32.

    The input packs [x | mids | iota] along the free dim so a single DMA
    (single semaphore) covers all compute dependencies — walrus allows only
    one sync-wait on TensorScalar instructions.
    """
    W = E + (K - 1) + K
    nc = bacc.Bacc()
    xmi_d = nc.declare_dram_parameter("xmi", [P, W], mybir.dt.float32, isOutput=False)
    out_d = nc.declare_dram_parameter("out", [P, E * K], mybir.dt.float32, isOutput=True)

    groups, chunks = _chunk_plan(E)

    with tile.TileContext(nc) as tc, ExitStack() as ctx:
        const = ctx.enter_context(tc.tile_pool(name="const", bufs=1))
        accp = ctx.enter_context(tc.tile_pool(name="acc", bufs=1))
        ohp = ctx.enter_context(tc.tile_pool(name="oh", bufs=4))

        xmi = const.tile([P, W], mybir.dt.float32, tag="xmi")
        nc.sync.dma_start(xmi[:], xmi_d[:])
        m = xmi[:, E : E + K - 1]
        iota = xmi[:, E + K - 1 : W]



        accs = {}
        for goff, glen in groups:
            acc = accp.tile([P, glen], mybir.dt.float32, tag=f"acc{goff}")
            accs[goff] = acc
            xg = xmi[:, goff : goff + glen]
            # first threshold initializes acc, remaining 62 accumulate
            nc.vector.tensor_scalar(
                out=acc[:], in0=xg, scalar1=m[:, 0:1], scalar2=None,
                op0=mybir.AluOpType.is_gt,
            )
            for k in range(1, K - 1):
                nc.vector.scalar_tensor_tensor(
                    out=acc[:], in0=xg, scalar=m[:, k : k + 1], in1=acc[:],
                    op0=mybir.AluOpType.is_gt, op1=mybir.AluOpType.add,
                )

        acc_locals = {}
        for coff, clen, eng in chunks:
            # find owning group
            goff, glen = next(g for g in groups if g[0] <= coff < g[0] + g[1])
            j0 = coff - goff
            oh = ohp.tile([P, clen * K], mybir.dt.float32, tag="oh")
            oh_view = oh[:].rearrange("p (e k) -> p e k", k=K)
            if eng == "v":
                acc_b = (
                    accs[goff][:, j0 : j0 + clen]
                    .unsqueeze(2)
                    .broadcast_to([P, clen, K])
                )
                iota_b = iota.unsqueeze(1).broadcast_to([P, clen, K])
                nc.vector.tensor_tensor(
                    out=oh_view, in0=acc_b, in1=iota_b, op=mybir.AluOpType.is_equal
                )
            else:
                if goff not in acc_locals:
                    al = accp.tile([P, glen], mybir.dt.float32, tag=f"accl{goff}")
                    nc.gpsimd.tensor_copy(al[:], accs[goff][:])
                    acc_locals[goff] = al
                al = acc_locals[goff]
                acc_b = (
                    al[:, j0 : j0 + clen].unsqueeze(2).broadcast_to([P, clen, K])
                )
                iota_b = iota_g[:].unsqueeze(1).broadcast_to([P, clen, K])
                # absorber: takes the oh-slot recycle (DMA-done) wait so the
                # is_equal itself carries only the single Pool-sem wait
                nc.gpsimd.memset(oh[0:1, 0:1], 0.0)
                nc.gpsimd.tensor_tensor(
                    out=oh_view, in0=acc_b, in1=iota_b, op=mybir.AluOpType.is_equal
                )
            nc.sync.dma_start(out_d[:, coff * K : (coff + clen) * K], oh[:])

    nc.compile()
    return nc


def _prep_host(y_true, mask, centers, t_keep):
    t_keep = int(t_keep)
    B, T, C, F = y_true.shape
    masktime = np.asarray(mask[0, :, 0, 0])
    keep_idx = np.argsort(masktime, kind="stable")[:t_keep]
    x = np.ascontiguousarray(np.asarray(y_true)[:, keep_idx])  # [B, t_keep, C, F]

    centers = np.asarray(centers)
    order = np.argsort(centers, kind="stable")
    cs = centers[order].astype(np.float64)
    mids = ((cs[:-1] + cs[1:]) / 2.0).astype(np.float32)  # [K-1]
    inv_order = np.empty(K, np.int64)
    inv_order[order] = np.arange(K)

    m_rep = np.ascontiguousarray(np.tile(mids, (P, 1)))
    iota_rep = np.ascontiguousarray(np.tile(inv_order.astype(np.float32), (P, 1)))
    return x, m_rep, iota_rep, t_keep


def _tie_fixups(x, centers, order, mids):
    """Indices where fp32 argmin tie-breaking differs from the interval rule.

    The interval method picks the lower sorted center at an exact fp32
    distance tie; jnp.argmin picks the smallest *original* index.  Ties only
    occur between the two centers flanking x, so checking sorted candidates
    {s-1, s, s+1} reproduces argmin exactly.  Returns (flat_idx, base, win).
    """
    xf = x.reshape(-1)
    s = np.searchsorted(mids, xf, side="left")  # rank in [0, K-1]
    cand = np.stack([np.clip(s - 1, 0, K - 1), s, np.clip(s + 1, 0, K - 1)])
    cand_orig = order[cand]  # [3, N] original center indices
    d = np.abs(xf[None, :] - centers[cand_orig]).astype(np.float32)
    dmin = d.min(axis=0)
    big = np.where(d == dmin, cand_orig, K)
    win = big.min(axis=0)  # argmin winner with original-index tiebreak
    base = order[s]  # what the device interval method picks
    bad = np.nonzero(win != base)[0]
    return bad, base[bad], win[bad]


def kernel(y_true, mask, centers, t_keep):
    global LAST_RESULTS
    y_true = np.asarray(y_true)
    B, T, C, F = y_true.shape
    x, m_rep, iota_rep, t_keep = _prep_host(y_true, mask, centers, t_keep)
    total = t_keep * C * F
    assert total % P == 0, (t_keep, C, F)
    E = total // P
    assert B == NCORES, B

    nc = _build(E)
    in_maps = [
        {
            "xmi": np.concatenate(
                [x[b].reshape(P, E), m_rep, iota_rep], axis=1
            )
        }
        for b in range(B)
    ]
    res = run_bass_kernel_spmd(nc, in_maps, list(range(NCORES)), trace=TRACE)
    LAST_RESULTS = res
    out = np.stack(
        [res.results[b]["out"].reshape(t_keep, C, F, K) for b in range(B)]
    )

    # exact fp32 argmin tie-break fixup (a handful of elements, if any)
    centers_np = np.asarray(centers)
    order = np.argsort(centers_np, kind="stable")
    cs = centers_np[order].astype(np.float64)
    mids = ((cs[:-1] + cs[1:]) / 2.0).astype(np.float32)
    bad, base, win = _tie_fixups(x, centers_np, order, mids)
    if bad.size:
        flat = out.reshape(-1, K)
        flat[bad, base] = 0.0
        flat[bad, win] = 1.0

    return out.astype(y_true.dtype, copy=False)
